# revision 50
# baseline (speedup 1.0000x reference)
"""AxialAttention Trainium2 kernel (8 NeuronCores, SPMD).

Sharding: core = b*4 + q; each core handles one batch element and a 10-row
H-slab with all 256 channels. The three reference "branches" are numerically
identical (h=w=d=40), so out = 3 * branch; the 3 is folded into wp and the
attention scale into wq/bq.

v3: all q/k/v pivot transposes run on the DMA engines' XBAR transpose
(InstDmaTransposeAnt) instead of the PE array, eliminating both the PE
transpose cost and the psum->SBUF evacuation traffic of the pivots.

Per H-slice, q/k are stored padded as [c, (w, deck*64 + d)] and v as
[c, (d, deck*64 + w)] (128-column chunks; 64-pitch pad). The XBAR transpose
semantics   out[p, 128j + c] = in[c, 128j + p]   then land deck-0 data at
partitions 0-39 and deck-1 at 64-103 -- both legal matmul partition bases.
Scores/AV run per (channel, deck) on 40x40 tiles read from the transposed
tiles via strided APs; softmax denominators come from a persistent ones
column appended to v (chunk 40 of each transposed slot).

conv/wp/AV psum tiles are double-wide ([128, 1024] f32 spanning 2 banks;
each matmul still targets a single bank) so one engine op drains two banks,
halving fixed per-op evacuation overheads.
"""

import sys

sys.path.insert(0, "/opt/trn_rl_repo")

import numpy as np
import ml_dtypes
from contextlib import ExitStack

import concourse.bass as bass
import concourse.tile as tile
from concourse import bacc, mybir
from concourse.bass_utils import run_bass_kernel_spmd
from concourse.masks import make_identity

BF16 = mybir.dt.bfloat16
F32 = mybir.dt.float32

B, C, H, W, D = 2, 256, 40, 40, 40
HEADS = 8
HD = C // HEADS
SCALE = HD ** -0.5
N_CORES = 8
SLAB = H // 4           # 10 H-rows per core
WD = W * D              # 1600
NSLAB = SLAB * WD       # 16000
PADC = W * 128          # 5120 padded cols per tensor per slice
GRP = 12                # channels per psum bank group
NGRP = (128 + GRP - 1) // GRP   # 11


PHASE_TRACE = {}   # inst name -> phase label (filled during build)


def _merge(a, b):
    """Proportionally interleave two chunk lists, preserving each order."""
    out = []
    na, nb = len(a), len(b)
    ia = ib = 0
    while ia < na or ib < nb:
        if ib >= nb or (ia * (nb + 1) <= ib * (na + 1) and ia < na):
            out.append(a[ia])
            ia += 1
        else:
            out.append(b[ib])
            ib += 1
    return out


def _build_nc():
    nc = bacc.Bacc(
        "TRN2",
        target_bir_lowering=False,
        debug=False,
        num_devices=N_CORES,
    )
    x_d = nc.declare_dram_parameter("x", [C, NSLAB], BF16, isOutput=False)
    wqkv_d = nc.declare_dram_parameter("wqkv", [C, 3 * C], BF16, isOutput=False)
    bqkv_d = nc.declare_dram_parameter("bqkv", [3 * C, 1], F32, isOutput=False)
    wp_d = nc.declare_dram_parameter("wp3", [C, C], BF16, isOutput=False)
    bp_d = nc.declare_dram_parameter("bp", [C, 1], F32, isOutput=False)
    out_d = nc.declare_dram_parameter("out", [C, NSLAB], F32, isOutput=True)

    IDENT = mybir.ActivationFunctionType.Identity
    EXP = mybir.ActivationFunctionType.Exp
    MULT = mybir.AluOpType.mult

    with ExitStack() as ctx:
        tc = ctx.enter_context(tile.TileContext(nc))
        const = ctx.enter_context(tc.tile_pool(name="const", bufs=1))
        xp = ctx.enter_context(tc.tile_pool(name="xp", bufs=3))
        padp = ctx.enter_context(tc.tile_pool(name="padp", bufs=2))
        ttp = ctx.enter_context(tc.tile_pool(name="ttp", bufs=8))
        oap = ctx.enter_context(tc.tile_pool(name="oap", bufs=2))
        brp = ctx.enter_context(tc.tile_pool(name="brp", bufs=1))
        ep = ctx.enter_context(tc.tile_pool(name="ep", bufs=3))
        recp = ctx.enter_context(tc.tile_pool(name="recp", bufs=4))
        # conv/scores/wp/pivot-back share one 4-deep 2KB tag; AV has its own
        ps_a = ctx.enter_context(tc.tile_pool(name="ps_a", bufs=4, space="PSUM"))
        ps_o = ctx.enter_context(tc.tile_pool(name="ps_o", bufs=2, space="PSUM"))
        ps_t = ctx.enter_context(tc.tile_pool(name="ps_t", bufs=2, space="PSUM"))

        ident = const.tile([128, 128], BF16)
        make_identity(nc, ident[:])

        wqkv_sb = const.tile([128, 2, 3 * C], BF16)
        wp_sb = const.tile([128, 2, C], BF16)
        bqkv_sb = const.tile([128, 6, 1], F32)
        bp_sb = const.tile([128, 2, 1], F32)

        def load_weights():
            nc.scalar.dma_start(
                wqkv_sb[:], wqkv_d.ap().rearrange("(ko ki) m -> ki ko m", ki=128)
            )
            nc.scalar.dma_start(
                bqkv_sb[:], bqkv_d.ap().rearrange("(mo mi) one -> mi mo one", mi=128)
            )
            nc.scalar.dma_start(
                wp_sb[:], wp_d.ap().rearrange("(ko ki) m -> ki ko m", ki=128)
            )
            nc.scalar.dma_start(
                bp_sb[:], bp_d.ap().rearrange("(mo mi) one -> mi mo one", mi=128)
            )

        branch = brp.tile([128, 2, WD], BF16)
        outp = ctx.enter_context(tc.tile_pool(name="outp", bufs=4))

        def load_x(i):
            x_sb = xp.tile([128, 2, WD], BF16, name="x_sb")
            nc.sync.dma_start(
                x_sb[:],
                x_d.ap()[:, i * WD : (i + 1) * WD].rearrange(
                    "(ko ki) n -> ki ko n", ki=128
                ),
            )
            return x_sb

        def tslot():
            # [128, 41, 128]; chunk 40 is the persistent ones column (set once
            # below; transpose DMAs only ever write chunks 0-39).
            return ttp.tile([128, 41, 128], BF16, tag="tt", name="tslot")

        evac_rr = [0]

        def evac(dst, src, bias=None):
            r = evac_rr[0] % 2
            evac_rr[0] += 1
            if bias is None:
                if r == 0:
                    nc.vector.tensor_copy(out=dst, in_=src)
                else:
                    nc.scalar.copy(dst, src)
            else:
                if r == 0:
                    nc.vector.tensor_scalar_add(dst, src, bias)
                else:
                    nc.scalar.activation(
                        out=dst, in_=src, func=IDENT, bias=bias, scale=1.0
                    )

        def conv_emit(xref, slots):
            """qkv conv of one slice into padded q/k/v, one transpose DMA per
            tensor. Double-wide chunks: (m-block, n-pair) -> 2 psum banks ->
            one evacuation of [128, 2x400]. xref is a 1-elem list holding the
            x tile (filled by the preceding load chunk)."""
            chunks = []
            pads = [None, None, None]

            for tn in range(3):
                def alloc_pad(tn=tn):
                    pads[tn] = padp.tile(
                        [128, PADC], BF16, tag=f"pad{tn}", name=f"pad{tn}"
                    )
                for deck in range(2):
                    m = 2 * tn + deck
                    for n in range(4):
                        def ch(tn=tn, deck=deck, m=m, n=n, alloc_pad=alloc_pad):
                            if pads[tn] is None:
                                alloc_pad()
                            pad = pads[tn]
                            ps = ps_a.tile(
                                [128, 512], F32, tag="ps_a", name="conv_ps"
                            )[:, 0:400]
                            for k in range(2):
                                nc.tensor.matmul(
                                    ps[:],
                                    lhsT=wqkv_sb[:, k, m * 128 : (m + 1) * 128],
                                    rhs=xref[0][:, k, n * 400 : (n + 1) * 400],
                                    start=(k == 0),
                                    stop=(k == 1),
                                )
                            w0 = 10 * n
                            ps_v = ps.rearrange("p (w d) -> p w d", d=40)
                            if tn < 2:
                                # q/k: col = w*128 + deck*64 + d
                                dst = pad.rearrange(
                                    "p (w x) -> p w x", x=128
                                )[:, w0 : w0 + 10, 64 * deck : 64 * deck + 40]
                            else:
                                # v: col = d*128 + deck*64 + w
                                dst = pad.rearrange(
                                    "p (d x) -> p x d", x=128
                                )[:, 64 * deck + w0 : 64 * deck + w0 + 10, :]
                            evac(dst, ps_v, bqkv_sb[:, m])
                        chunks.append(ch)

                def dma_ch(tn=tn):
                    t = tslot()
                    slots[tn] = t
                    nc.sync.dma_start(t[:, 0:40, :], pads[tn][:], transpose=True)
                chunks.append(dma_ch)
            return chunks

        def attn_emit(slots):
            """scores -> exp -> AV per 12-channel group; AV psum double-wide,
            evacuated per group-pair into o_all."""
            o_all = oap.tile([128, 128 * W], BF16, name="o_all")

            def scores_stage(g):
                c0 = g * GRP
                gn = min(GRP, 128 - c0)
                s_ps = ps_a.tile([128, 512], F32, tag="ps_a", name="s_ps")
                for j in range(gn):
                    c = c0 + j
                    for dk in range(2):
                        r = 64 * dk
                        nc.tensor.matmul(
                            s_ps[r : r + 40, j * 40 : (j + 1) * 40],
                            lhsT=slots[1][r : r + 40, 0:40, c],
                            rhs=slots[0][r : r + 40, 0:40, c],
                            start=True,
                            stop=True,
                        )
                e_sb = ep.tile([128, 480], BF16, tag="e_sb", name="e_sb")
                nc.scalar.activation(
                    out=e_sb[0:104, : gn * 40],
                    in_=s_ps[0:104, : gn * 40],
                    func=EXP,
                )
                return e_sb

            def av_stage(g, e_sb):
                c0 = g * GRP
                gn = min(GRP, 128 - c0)
                o_ps = ps_o.tile([128, 512], F32, tag="ps_o", name="o_ps")
                for j in range(gn):
                    c = c0 + j
                    for dk in range(2):
                        r = 64 * dk
                        nc.tensor.matmul(
                            o_ps[r : r + 41, j * 40 : (j + 1) * 40],
                            lhsT=slots[2][r : r + 40, 0:41, c],
                            rhs=e_sb[r : r + 40, j * 40 : (j + 1) * 40],
                            start=True,
                            stop=True,
                        )
                evac(
                    o_all[0:105, c0 * 40 : (c0 + gn) * 40],
                    o_ps[0:105, : gn * 40],
                )

            pend = [None]
            chunks = []
            for g in range(NGRP):
                def ch(g=g):
                    e_sb = scores_stage(g)
                    if pend[0] is not None:
                        av_stage(*pend[0])
                    pend[0] = (g, e_sb)
                chunks.append(ch)
            chunks.append(lambda: av_stage(*pend[0]))
            return o_all, chunks

        def pbwp_emit(o_all, i):
            """pivot-back + normalize + wp conv + out DMA for slice i.
            Out-DMA issues are returned separately: issuing them inline would
            head-of-line block the SP sequencer (their dep chains clear late)
            and delay the transpose DMA issues queued behind them."""
            o_v = o_all.rearrange("p (c w) -> p w c", w=W)
            chunks = []
            wp_chunks = []
            dma_chunks = []
            for wb in range(5):
                def ch(wb=wb):
                    w0 = 8 * wb
                    pb_full = ps_t.tile([128, 1024], BF16, tag="ps_t", name="pb")
                    pb = pb_full[:, 0:848]
                    for wl in range(8):
                        w = w0 + wl
                        nc.tensor.transpose(
                            pb[:, wl * 106 : wl * 106 + 105],
                            o_v[0:105, w, :],
                            ident[0:105, 0:105],
                        )
                    pb_v = pb.rearrange("p (w q) -> p w q", q=106)
                    rec = recp.tile([128, 8, 2], F32, tag="rec", name="rec")
                    nc.vector.reciprocal(rec[:, :, 0], pb_v[:, :, 40])
                    nc.vector.reciprocal(rec[:, :, 1], pb_v[:, :, 104])
                    for dk in range(2):
                        nc.vector.tensor_tensor(
                            branch[:, dk].rearrange("p (w d) -> p w d", d=40)[
                                :, w0 : w0 + 8
                            ],
                            pb_v[:, :, 64 * dk : 64 * dk + 40],
                            rec[:, :, dk : dk + 1].to_broadcast((128, 8, 40)),
                            MULT,
                        )
                chunks.append(ch)
            # wp: quarter-slice granularity, double-buffered out staging;
            # wp psum shares the pivot-back tag (not conv/scores') so a slow
            # out drain can't stall the main conv/scores psum rotation.
            for n in range(4):
                oq = [None]
                for m in range(2):
                    def ch(n=n, m=m, oq=oq):
                        if oq[0] is None:
                            oq[0] = outp.tile(
                                [128, 2, 400], F32, tag="oq", name="oq"
                            )
                        ps = ps_t.tile(
                            [128, 1024], BF16, tag="ps_t", name="wp_ps"
                        ).bitcast(F32)[:, 0:400]
                        for k in range(2):
                            nc.tensor.matmul(
                                ps[:],
                                lhsT=wp_sb[:, k, m * 128 : (m + 1) * 128],
                                rhs=branch[:, k, n * 400 : (n + 1) * 400],
                                start=(k == 0),
                                stop=(k == 1),
                            )
                        evac(oq[0][:, m], ps[:], bp_sb[:, m])
                    wp_chunks.append(ch)

                def dma_ch(n=n, i=i, oq=oq):
                    nc.sync.dma_start(
                        out_d.ap()[
                            :, i * WD + n * 400 : i * WD + (n + 1) * 400
                        ].rearrange("(ko ki) n -> ki ko n", ki=128),
                        oq[0][:],
                    )
                dma_chunks.append(dma_ch)
            return chunks, wp_chunks, dma_chunks

        # ones columns for AV denominators: chunk 40 of each of the 8 slots.
        first = [tslot() for _ in range(8)]
        for t in first:
            nc.vector.memset(t[:, 40, :], 1.0)
        del first  # pool rotation reuses these slots

        def run(chs, label):
            for ch in chs:
                before = len(nc.inst_map)
                ch()
                lbl = getattr(ch, "_label", label)
                for nm in list(nc.inst_map.keys())[before:]:
                    PHASE_TRACE[nm] = lbl

        def lab(chs, label):
            for j, ch in enumerate(chs):
                ch._label = f"{label}.{j}"
            return chs

        # ---- software-pipelined slice loop (3 stages deep)
        # conv runs TWO slices ahead of attention so the transpose DMAs of
        # slice i complete long before scores(i) need them; x loads run one
        # further slice ahead so conv never head-of-line blocks the PE queue.
        load_weights()
        xrefs = {j: [load_x(j)] for j in range(min(3, SLAB))}
        slots = {0: [None, None, None]}
        run(conv_emit(xrefs[0], slots[0]), "conv0")
        if SLAB > 1:
            slots[1] = [None, None, None]
            run(conv_emit(xrefs[1], slots[1]), "conv1")
        pending_pb, pending_wp, pending_odma = [], [], []
        for i in range(SLAB):
            evac_rr[0] = 0
            o_all, attn_ch = attn_emit(slots.pop(i))
            lab(attn_ch, f'attn{i}')
            conv_chunks = []
            if i + 2 < SLAB:
                if i + 3 < SLAB:
                    xrefs[i + 3] = xr = [None]
                    def ch_load(i=i, xr=xr):
                        xr[0] = load_x(i + 3)
                    conv_chunks.append(ch_load)
                slots[i + 2] = [None, None, None]
                conv_chunks += lab(conv_emit(xrefs.pop(i + 2), slots[i + 2]), f'conv{i+2}')
            # pbwp(i-1) runs unmerged first: its DVE mults must queue ahead
            # of attn(i)'s o-evacs, or the wp matmuls stall behind AV
            # (DVE executes in order).
            run(pending_pb, f"s{i}")
            run(pending_wp, f"wpf{i}")
            run(_merge(attn_ch, conv_chunks), f"s{i}")
            run(pending_odma, f"odma{i - 1}")
            pending_pb, pending_wp, pending_odma = pbwp_emit(o_all, i)
            lab(pending_pb, f'pbwp{i}')
            lab(pending_wp, f'wp{i}')
        run(pending_pb, "pbwp_last")
        run(pending_wp, "wp_last")
        run(pending_odma, "odma_last")

    nc.compile()
    return nc


_NC_CACHE = None


def _get_nc():
    global _NC_CACHE
    if _NC_CACHE is None:
        _NC_CACHE = _build_nc()
    return _NC_CACHE


def make_in_maps(x, wq, bq, wk, bk, wv, bv, wp, bp):
    bf = ml_dtypes.bfloat16
    wqkv = np.concatenate(
        [wq.T * SCALE, wk.T, wv.T], axis=1
    ).astype(bf)  # [C, 3C], lhsT layout (c_in rows, c_out cols)
    bqkv = np.concatenate([bq * SCALE, bk, bv]).reshape(3 * C, 1).astype(np.float32)
    wp3 = (3.0 * wp).T.astype(bf)  # [C, C]
    bp_ = bp.reshape(C, 1).astype(np.float32)
    in_maps = []
    for core in range(N_CORES):
        b = core // 4
        r0 = (core % 4) * SLAB
        x_slab = np.ascontiguousarray(
            x[b, :, r0 : r0 + SLAB].reshape(C, NSLAB)
        ).astype(bf)
        in_maps.append(
            {"x": x_slab, "wqkv": wqkv, "bqkv": bqkv, "wp3": wp3, "bp": bp_}
        )
    return in_maps


def run_on_cores(in_maps, **kw):
    nc = _get_nc()
    return run_bass_kernel_spmd(nc, in_maps, core_ids=list(range(N_CORES)), **kw)


def kernel(x, wq, bq, wk, bk, wv, bv, wp, bp):
    x = np.asarray(x, dtype=np.float32)
    in_maps = make_in_maps(
        x,
        np.asarray(wq, np.float32),
        np.asarray(bq, np.float32),
        np.asarray(wk, np.float32),
        np.asarray(bk, np.float32),
        np.asarray(wv, np.float32),
        np.asarray(bv, np.float32),
        np.asarray(wp, np.float32),
        np.asarray(bp, np.float32),
    )
    res = run_on_cores(in_maps)
    out = np.empty((B, C, H, W, D), np.float32)
    for core in range(N_CORES):
        b = core // 4
        r0 = (core % 4) * SLAB
        out[b, :, r0 : r0 + SLAB] = res.results[core]["out"].reshape(C, SLAB, W, D)
    return out


if __name__ == "__main__":
    rng = np.random.default_rng(0)
    ins = {
        "x": rng.standard_normal((B, C, H, W, D), np.float32),
        "wq": rng.standard_normal((C, C), np.float32) / 16,
        "bq": rng.standard_normal(C).astype(np.float32) * 0.01,
        "wk": rng.standard_normal((C, C), np.float32) / 16,
        "bk": rng.standard_normal(C).astype(np.float32) * 0.01,
        "wv": rng.standard_normal((C, C), np.float32) / 16,
        "bv": rng.standard_normal(C).astype(np.float32) * 0.01,
        "wp": rng.standard_normal((C, C), np.float32) / 16,
        "bp": rng.standard_normal(C).astype(np.float32) * 0.01,
    }
    out = kernel(**ins)
    print("kernel ran, out shape", out.shape, "mean", float(np.abs(out).mean()))
